# revision 7
# baseline (speedup 1.0000x reference)
"""Trainium2 Bass kernel for nn_CrossModalFusionModel (sparse sliding-window
cross-attention, 2 modules: image<-text and text<-image).

Sharding: head-parallel tensor parallelism over 8 NeuronCores. Core h owns
attention head h (dh=128) of BOTH modules: it computes its head's Q/K/V with
host-folded projection weights (input-proj and attention-proj chains collapse
into one matmul), runs banded attention for that head, and emits a full-D
o-projection partial plus its D-slice of the residual projection. The host
sums the 8 partials (the unshard step). No collectives.

Cost-model-driven structure (v2 cost model):
- DMA issue is expensive (625ns HWDGE + 650ns SEQ hold per DMA, all DMAs
  serialize on one HWDGE device and one DMA_ENGINES device): inputs are
  packed into 4 dram tensors loaded by ~6 slice-DMAs on the SP queue, small
  consts ride the Pool SWDGE queue.
- po partials are emitted in fp8e4 (x16 scale): halves output bytes; the
  8-way host sum tolerates the quantization.
- Residual is a single hi-fp8 pass (x16-scaled weights); the lo-term
  corrections are dropped (error budget covers it).
- V is projected directly into [key, dh] layout with narrow DoubleRow
  matmuls per key-tile (no PE transposes); the v-bias moves to the host
  output bias since softmax weights sum to 1 (zero-pad slots included).
- ssum/AV accumulation opens PSUM with disjoint start=True groups
  (c0 covers q[0:256), c3 covers q[256:512)): no opener matmuls.
- oproj matmul pairs write f16 PSUM banks [128, 2N]; single wide copies
  (balanced across Act/DVE/Pool) cast to fp8 stages that stream out from
  mid-kernel instead of piling into a tail.
- PE warmup matmuls seeded from an on-chip memset burn the cost model's
  pe-ramp window while the first DMAs land.
"""

import math

import numpy as np
import ml_dtypes

N = 512          # tokens / patches
DM = 1024        # d_model
DH = 128         # head dim
NT = N // 128    # 4 tiles
C_IMG = 1024
C_TXT = 768
WINDOW = 64
NCORES = 8

# per key-tile c, the consecutive query tiles it serves
CQTS = [[0, 1], [0, 1, 2], [1, 2, 3], [2, 3]]
GW = [len(q) * 128 for q in CQTS]            # group widths
GBASE = np.cumsum([0] + GW).tolist()
NG = GBASE[-1]                               # 1280

COMPUTE_DTYPE = "f16"
WARMUP_MM = 7
FP8_SCALE = 16.0    # folded into wq/wk/wv/rw (unfolded via Exp scale / host)
PO_SCALE = 16.0     # extra scale on fp8 po partials (folded into wo)

_prog_cache = {}
LAST_RESULT = {}

# cpack layout (CD [128, CP_LEN]); col-major consts then partition-0 rows
CP_MASK3 = 0                       # [128, 384] band masks [d+1 | d0 | d-1]
CP_COLC = 384                      # kbc_ia | kbc_ta | ones_col
CP_MASKP = 387                     # row [1, 512] at partition 0: npad counts
CP_ONESR = CP_MASKP + N            # row [1, 128] of ones
CP_LEN = CP_ONESR + DH
# tC layout (CD [128, TC_LEN]): cpack | wo_ia | wo_ta
TC_WO = {"ia": CP_LEN, "ta": CP_LEN + DM}
TC_LEN = CP_LEN + 2 * DM
# pkA (fp8 [128, A_LEN]): wq_ta | xt8 | wk_ia | wv_ia
A_WQ_TA, A_XT8, A_WK_IA, A_WV_IA = 0, 768, 3840, 4608
A_LEN = 5376
A_SPLIT = 1792            # first DMA: wq_ta + xt8 chunks 0-1
# pkB (fp8 [128, B_LEN]): wq_ia | xi8 | wk_ta | wv_ta
B_WQ_IA, B_XI8, B_WK_TA, B_WV_TA = 0, 1024, 5120, 6144
B_LEN = 7168
B_SPLIT = 3072            # first DMA: wq_ia + xi8 chunks 0-3
# pkD (fp8 [128, D_LEN]): rwh_i | rwh_t
D_RWH_I, D_RWH_T = 0, 1024
D_LEN = 1792
# colf (f32 [128, 8]): bq_ia bk_ia bq_ta bk_ta brx brt - -


def _np_cd(cd):
    return {"f16": np.float16, "bf16": ml_dtypes.bfloat16}[cd]


def _host_cd(x, cd):
    return np.ascontiguousarray(np.ascontiguousarray(x).astype(_np_cd(cd)))


def _q8(x):
    return np.ascontiguousarray(
        np.asarray(x, dtype=np.float32).astype(ml_dtypes.float8_e4m3))


def _pm(w):
    """[C, X] -> partition-major [128, (C//128)*X]."""
    C, X = w.shape
    return (np.ascontiguousarray(w).reshape(C // 128, 128, X)
            .transpose(1, 0, 2).reshape(128, (C // 128) * X))


def _build_program(cd):
    import concourse.bass as bass
    import concourse.tile as tile
    from concourse import bacc, mybir

    f32 = mybir.dt.float32
    CD = {"f16": mybir.dt.float16, "bf16": mybir.dt.bfloat16}[cd]
    CD8 = mybir.dt.float8e4
    Exp = mybir.ActivationFunctionType.Exp
    Copy = mybir.ActivationFunctionType.Copy
    Ident = mybir.ActivationFunctionType.Identity
    DR = mybir.MatmulPerfMode.DoubleRow

    nc = bacc.Bacc("TRN2", target_bir_lowering=False, debug=False,
                   num_devices=NCORES)

    d_pkA = nc.dram_tensor("pkA", [128, A_LEN], CD8, kind="ExternalInput")
    d_pkB = nc.dram_tensor("pkB", [128, B_LEN], CD8, kind="ExternalInput")
    d_pkC = nc.dram_tensor("pkC", [128, TC_LEN], CD, kind="ExternalInput")
    d_pkD = nc.dram_tensor("pkD", [128, D_LEN], CD8, kind="ExternalInput")
    d_colf = nc.dram_tensor("colf", [128, 8], f32, kind="ExternalInput")

    d_po = {m: nc.dram_tensor(f"po_{m}", [DM, N], CD8, kind="ExternalOutput")
            for m in ("ia", "ta")}
    d_xr = nc.dram_tensor("xr", [128, 2, N], CD, kind="ExternalOutput")

    DESCALE = 1.0 / (FP8_SCALE * FP8_SCALE)

    with tile.TileContext(nc) as tc:
        with tc.tile_pool(name="consts", bufs=1) as consts, \
             tc.tile_pool(name="work", bufs=2) as work, \
             tc.tile_pool(name="ps_big", bufs=4, space="PSUM") as ps_big, \
             tc.tile_pool(name="ps_po", bufs=2, space="PSUM") as ps_po:

            # ---- PE warmup seeded from an on-chip memset (no DMA dep).
            # The dummy Exp forces the ACT table load (1283ns) into the
            # idle startup window.
            seed = consts.tile([128, N], CD, tag="seed")
            nc.gpsimd.memset(seed[:], 0.0)
            dummy = consts.tile([1, N], CD, tag="dummy")
            nc.scalar.activation(dummy[:], seed[0:1, :], Exp)
            warm_ps = ps_big.tile([128, N], f32, tag="big", name="warm_ps")
            for _ in range(WARMUP_MM):
                nc.tensor.matmul(warm_ps[:], seed[0:1, 0:128], dummy[:],
                                 start=True, stop=True)

            # ---- DMAs: packed inputs, slice loads in dependency order ----
            tA = consts.tile([128, A_LEN], CD8, tag="tA", name="tA")
            tB = consts.tile([128, B_LEN], CD8, tag="tB", name="tB")
            tC = consts.tile([128, TC_LEN], CD, tag="tC", name="tC")
            tD = consts.tile([128, D_LEN], CD8, tag="tD", name="tD")
            colf = consts.tile([128, 8], f32, tag="colf")

            nc.sync.dma_start(tA[:, 0:A_SPLIT], d_pkA[:, 0:A_SPLIT])
            nc.sync.dma_start(tA[:, A_SPLIT:], d_pkA[:, A_SPLIT:])
            nc.sync.dma_start(tC[:, 0:CP_LEN], d_pkC[:, 0:CP_LEN])
            nc.sync.dma_start(tB[:, 0:B_SPLIT], d_pkB[:, 0:B_SPLIT])
            nc.sync.dma_start(tB[:, B_SPLIT:], d_pkB[:, B_SPLIT:])
            nc.sync.dma_start(tC[:, CP_LEN:], d_pkC[:, CP_LEN:])
            nc.gpsimd.dma_start(colf[:], d_colf[:])
            nc.gpsimd.dma_start(tD[:], d_pkD[:])

            def c3(t, off, nct):
                return t[:, off:off + nct * 128].rearrange(
                    "p (c x) -> p c x", x=128)

            def x3(t, off, nct):
                return t[:, off:off + nct * N].rearrange(
                    "p (c x) -> p c x", x=N)

            wq = {"ia": c3(tB, B_WQ_IA, 8), "ta": c3(tA, A_WQ_TA, 6)}
            wk = {"ia": c3(tA, A_WK_IA, 6), "ta": c3(tB, B_WK_TA, 8)}
            wv = {"ia": c3(tA, A_WV_IA, 6), "ta": c3(tB, B_WV_TA, 8)}
            xt8 = x3(tA, A_XT8, 6)
            xi8 = x3(tB, B_XI8, 8)
            rwh = {"i": c3(tD, D_RWH_I, 8), "t": c3(tD, D_RWH_T, 6)}
            wo = {m: tC[:, TC_WO[m]:TC_WO[m] + DM] for m in ("ia", "ta")}
            mask3 = tC[:, CP_MASK3:CP_MASK3 + 384]
            kbc = {"ia": tC[:, CP_COLC:CP_COLC + 1],
                   "ta": tC[:, CP_COLC + 1:CP_COLC + 2]}
            ones_col = tC[:, CP_COLC + 2:CP_COLC + 3]
            maskP = tC[0:1, CP_MASKP:CP_MASKP + N]
            ones_row = tC[0:1, CP_ONESR:CP_ONESR + DH]
            bq = {"ia": colf[:, 0:1], "ta": colf[:, 2:3]}
            bk = {"ia": colf[:, 1:2], "ta": colf[:, 3:4]}
            brx = colf[:, 4:5]
            brt = colf[:, 5:6]

            st = {"ia": {}, "ta": {}}

            def projT(m, which, w3, xx, nct, tag, evac):
                """[128, N] = (w^T x)^T via fp8 DR chunks; bias in evac.
                evac: 'a' = Act Ident+bias, 'v' = DVE tensor_scalar_add."""
                ps = ps_big.tile([128, N], f32, tag="big")
                for c in range(nct // 2):
                    nc.tensor.matmul(ps[:], w3[:, 2 * c:2 * c + 2, :],
                                     xx[:, 2 * c:2 * c + 2, :],
                                     perf_mode=DR, start=(c == 0),
                                     stop=(c == nct // 2 - 1))
                out = work.tile([128, N], CD, tag=tag, bufs=1, name=tag)
                col = bq[m] if which == "q" else bk[m]
                if evac == "a":
                    nc.scalar.activation(out[:], ps[:], Ident, bias=col)
                else:
                    nc.vector.tensor_scalar_add(out[:], ps[:], col[:])
                st[m][which] = out
                return out

            def vproj(m, xx, nct):
                """vN [key%128, c*DH+dh] direct: narrow DR matmuls per
                key-tile into one f16 PSUM tile; no bias (host-folded)."""
                vps = ps_big.tile([128, NT * DH], f32, tag="big", name="vps")
                for c in range(NT):
                    for j in range(nct // 2):
                        nc.tensor.matmul(
                            vps[:, c * DH:(c + 1) * DH],
                            xx[:, 2 * j:2 * j + 2, c * 128:(c + 1) * 128],
                            wv[m][:, 2 * j:2 * j + 2, :],
                            perf_mode=DR, start=(j == 0),
                            stop=(j == nct // 2 - 1))
                vN = work.tile([128, NT * DH], CD, tag=f"vN_{m}", bufs=1,
                               name=f"vN_{m}")
                nc.vector.tensor_copy(vN[:], vps[:])
                st[m]["vN"] = vN

            def attn_scores(m):
                """Banded scores by key-tile group -> exp -> mask."""
                qT, kT = st[m]["q"], st[m]["k"]
                eTm = work.tile([128, NG], CD, tag=f"eTm_{m}", bufs=1,
                                name=f"eTm_{m}")
                eT = work.tile([128, NG], CD, tag=f"eT_{m}", bufs=1,
                               name=f"eT_{m}")
                for c in range(NT):
                    qts = CQTS[c]
                    gps = ps_big.tile([128, GW[c]], f32, tag="big", name="gps")
                    nc.tensor.matmul(
                        gps[:], kT[:, c * 128:(c + 1) * 128],
                        qT[:, qts[0] * 128:(qts[-1] + 1) * 128],
                        start=True, stop=True)
                    sl = slice(GBASE[c], GBASE[c + 1])
                    nc.scalar.activation(eT[:, sl], gps[:], Exp,
                                         scale=DESCALE)
                    moff = (1 - (c - qts[0])) * 128
                    nc.vector.tensor_mul(eTm[:, sl], eT[:, sl],
                                         mask3[:, moff:moff + GW[c]])
                sp = ps_big.tile([128, N], f32, tag="big", name="sp")[0:1, :]
                nc.tensor.matmul(sp[:], kbc[m], qT[:], start=True, stop=True)
                eP = work.tile([1, N], CD, tag=f"eP_{m}", bufs=1,
                               name=f"eP_{m}")
                nc.scalar.activation(eP[:], sp[:], Exp, scale=DESCALE)
                ePm = work.tile([1, N], CD, tag=f"ePm_{m}", bufs=1,
                                name=f"ePm_{m}")
                nc.vector.tensor_mul(ePm[:], eP[:], maskP)
                st[m].update(eTm=eTm, ePm=ePm)

            # disjoint start groups: c0 covers q[0:256), c3 covers [256:512)
            GORDER = [0, 3, 1, 2]

            def attn_ssum(m):
                eTm, ePm = st[m]["eTm"], st[m]["ePm"]
                ssum = ps_big.tile([128, N], f32, tag="big", name="ssum")[0:1, :]
                for i, c in enumerate(GORDER):
                    qts = CQTS[c]
                    nc.tensor.matmul(
                        ssum[:, qts[0] * 128:(qts[-1] + 1) * 128],
                        ones_col, eTm[:, GBASE[c]:GBASE[c + 1]],
                        start=(i < 2), stop=False, skip_group_check=True)
                nc.tensor.matmul(ssum[:], ones_col[0:1, :], ePm[:],
                                 start=False, stop=True, skip_group_check=True)
                rinv = work.tile([1, N], CD, tag=f"rinv_{m}", bufs=1,
                                 name=f"rinv_{m}")
                with nc.allow_low_precision(
                        reason="1/denom feeds a 16-bit matmul; denom O(10-100)"):
                    nc.vector.reciprocal(rinv[:], ssum[:])
                st[m]["rinv"] = rinv

            def attn_rbc(m):
                rps = ps_big.tile([128, N], f32, tag="big", name="rps")
                nc.tensor.matmul(rps[:], ones_row, st[m]["rinv"][:],
                                 start=True, stop=True)
                rbc = work.tile([128, N], CD, tag=f"rbc_{m}", bufs=1,
                                name=f"rbc_{m}")
                nc.gpsimd.tensor_copy(rbc[:], rps[:])
                st[m]["rbc"] = rbc

            def attn_av(m):
                """oT [128, N] = V^T E^T; pad slots contribute zero (v-bias
                host-folded), so no pad matmul."""
                eTm, vN = st[m]["eTm"], st[m]["vN"]
                oT = ps_big.tile([128, N], f32, tag="big", name="oT")
                for i, c in enumerate(GORDER):
                    qts = CQTS[c]
                    nc.tensor.matmul(
                        oT[:, qts[0] * 128:(qts[-1] + 1) * 128],
                        vN[:, c * DH:(c + 1) * DH],
                        eTm[:, GBASE[c]:GBASE[c + 1]],
                        start=(i < 2), stop=(i == NT - 1),
                        skip_group_check=True)
                onorm = work.tile([128, N], CD, tag=f"onorm_{m}", bufs=1,
                                  name=f"onorm_{m}")
                nc.vector.tensor_mul(onorm[:], oT[:], st[m]["rbc"][:])
                st[m]["onorm"] = onorm

            def oproj_pair(m, p, eng):
                """chunks 2p,2p+1 -> one f16 PSUM bank -> one wide copy into
                the fp8 stage tile. eng in {'v','a','p'}."""
                onorm = st[m]["onorm"]
                pps = ps_po.tile([128, 2 * N], f32, tag="po", name="pps")
                for k in range(2):
                    dt_i = 2 * p + k
                    nc.tensor.matmul(pps[:, k * N:(k + 1) * N],
                                     wo[m][:, dt_i * 128:(dt_i + 1) * 128],
                                     onorm[:], start=True, stop=True)
                half = p // 2
                stage = st[m]["stage"][half]
                dst = stage[:, (p % 2) * 2:(p % 2) * 2 + 2, :].rearrange(
                    "p a b -> p (a b)")
                if eng == "v":
                    nc.vector.tensor_copy(dst, pps[:])
                elif eng == "a":
                    nc.scalar.activation(dst, pps[:], Copy)
                else:
                    nc.gpsimd.tensor_copy(dst, pps[:])

            def po_flush(m, half):
                stage = st[m]["stage"][half]
                nc.sync.dma_start(
                    d_po[m].ap().rearrange("(c p) n -> p c n", p=128)
                    [:, half * 4:half * 4 + 4, :], stage[:])

            for m in ("ia", "ta"):
                st[m]["stage"] = [
                    work.tile([128, 4, N], CD8, tag=f"stg_{m}{h}", bufs=1,
                              name=f"stg_{m}{h}") for h in range(2)]
            xr_sb = work.tile([128, 2, N], CD, tag="xr_sb", bufs=1,
                              name="xr_sb")

            def resid(which, xx, nct, ps):
                """resid*16 = xh@wh single hi-fp8 pass."""
                w3 = rwh[which]
                for c in range(nct // 2):
                    nc.tensor.matmul(ps[:], w3[:, 2 * c:2 * c + 2, :],
                                     xx[:, 2 * c:2 * c + 2, :], perf_mode=DR,
                                     start=(c == 0), stop=(c == nct // 2 - 1))

            # ---- schedule (program order == per-engine issue order) ----
            projT("ta", "q", wq["ta"], xt8, 6, "qta", "v")
            projT("ia", "k", wk["ia"], xt8, 6, "kia", "a")
            vproj("ia", xt8, 6)
            rx_t = ps_big.tile([128, N], f32, tag="big", name="rx_t")
            resid("t", xt8, 6, rx_t)
            nc.scalar.activation(xr_sb[:, 1, :], rx_t[:], Ident, bias=brt,
                                 scale=1.0 / FP8_SCALE)
            projT("ia", "q", wq["ia"], xi8, 8, "qia", "v")
            attn_scores("ia")
            projT("ta", "k", wk["ta"], xi8, 8, "kta", "a")
            vproj("ta", xi8, 8)
            rx_i = ps_big.tile([128, N], f32, tag="big", name="rx_i")
            resid("i", xi8, 8, rx_i)
            nc.scalar.activation(xr_sb[:, 0, :], rx_i[:], Ident, bias=brx,
                                 scale=1.0 / FP8_SCALE)
            nc.sync.dma_start(d_xr[:], xr_sb[:])
            attn_scores("ta")
            attn_ssum("ia")
            attn_rbc("ia")
            attn_ssum("ta")
            attn_rbc("ta")
            attn_av("ia")
            oproj_pair("ia", 0, "a")
            attn_av("ta")
            oproj_pair("ia", 1, "v")
            po_flush("ia", 0)
            oproj_pair("ta", 0, "p")
            oproj_pair("ia", 2, "a")
            oproj_pair("ta", 1, "v")
            oproj_pair("ia", 3, "p")
            po_flush("ia", 1)
            po_flush("ta", 0)
            oproj_pair("ta", 2, "a")
            oproj_pair("ta", 3, "v")
            po_flush("ta", 1)

    nc.compile()
    return nc


def _band_masks():
    """[128, 3*128] = [d+1 | d0 | d-1]; pattern for key tile c vs query
    tile qt is delta = c - qt, stored so a key-group's consecutive query
    tiles read one contiguous slice."""
    jj = np.arange(128)[:, None]
    ii = np.arange(128)[None, :]
    out = np.zeros((128, 3 * 128), dtype=np.float64)
    for i, d in enumerate((1, 0, -1)):
        delta = 128 * d + jj - ii
        out[:, i * 128:(i + 1) * 128] = ((delta >= -WINDOW // 2)
                                         & (delta <= WINDOW // 2 + 1))
    return out


def _npad():
    i = np.arange(N)
    lo = np.maximum(0, i - WINDOW // 2)
    hi = np.minimum(N - 1, i + WINDOW // 2 + 1)
    length = hi - lo + 1
    return np.maximum(0, WINDOW - length).astype(np.float64)


def kernel(**inputs):
    from concourse.bass_utils import run_bass_kernel_spmd

    cd = COMPUTE_DTYPE
    if cd not in _prog_cache:
        _prog_cache[cd] = _build_program(cd)
    nc = _prog_cache[cd]

    f8 = lambda x: np.asarray(x, dtype=np.float64)
    images = f8(inputs["images"])[0]        # [N, 1024]
    caps = f8(inputs["capitions"])[0]       # [N, 768]
    ip_w, ip_b = f8(inputs["ip_w"]), f8(inputs["ip_b"])
    tp_w, tp_b = f8(inputs["tp_w"]), f8(inputs["tp_b"])

    sc = 1.0 / math.sqrt(DH)
    s8 = FP8_SCALE
    band = _band_masks()
    xi8 = _q8(_pm(images.T))                # [128, 8*N]
    xt8 = _q8(_pm(caps.T))                  # [128, 6*N]

    in_maps = []
    ob_eff = {}
    for h in range(NCORES):
        sl = slice(h * DH, (h + 1) * DH)
        pkA = np.zeros((128, A_LEN), dtype=np.float64)
        pkB = np.zeros((128, B_LEN), dtype=np.float64)
        pkC = np.zeros((128, TC_LEN), dtype=np.float64)
        pkD = np.zeros((128, D_LEN), dtype=np.float64)
        colf = np.zeros((128, 8), dtype=np.float64)

        pkC[:, CP_MASK3:CP_MASK3 + 384] = band
        pkC[:, CP_COLC + 2] = 1.0
        pkC[0, CP_MASKP:CP_MASKP + N] = _npad()
        pkC[0, CP_ONESR:CP_ONESR + DH] = 1.0
        colf[:, 4] = ip_b[sl]
        colf[:, 5] = tp_b[sl]

        for mi, (m, pw, pb, cw, cb) in enumerate(
                (("ia", ip_w, ip_b, tp_w, tp_b),
                 ("ta", tp_w, tp_b, ip_w, ip_b))):
            qw, qb = f8(inputs[f"{m}_qw"]), f8(inputs[f"{m}_qb"])
            kw, kb = f8(inputs[f"{m}_kw"]), f8(inputs[f"{m}_kb"])
            vw, vb = f8(inputs[f"{m}_vw"]), f8(inputs[f"{m}_vb"])
            ow = f8(inputs[f"{m}_ow"])
            wq_p = _pm(((qw[sl] @ pw) * sc * s8).T)
            wk_p = _pm(((kw[sl] @ cw) * s8).T)
            wv_p = _pm(((vw[sl] @ cw) * s8).T)
            if m == "ia":
                pkA[:, A_WK_IA:A_WK_IA + 768] = wk_p
                pkA[:, A_WV_IA:A_WV_IA + 768] = wv_p
                pkB[:, B_WQ_IA:B_WQ_IA + 1024] = wq_p
            else:
                pkA[:, A_WQ_TA:A_WQ_TA + 768] = wq_p
                pkB[:, B_WK_TA:B_WK_TA + 1024] = wk_p
                pkB[:, B_WV_TA:B_WV_TA + 1024] = wv_p
            pkC[:, TC_WO[m]:TC_WO[m] + DM] = ow[:, sl].T * (PO_SCALE / s8)
            colf[:, 2 * mi] = (qw[sl] @ pb + qb[sl]) * sc * s8
            colf[:, 2 * mi + 1] = (kw[sl] @ cb + kb[sl]) * s8
            pkC[:, CP_COLC + mi] = kb[sl] * s8
            if h == 0:
                # v-bias folds into the output bias (softmax weights sum
                # to 1, zero-pad slots included)
                bvfull = vw @ cb + vb
                ob_eff[m] = f8(inputs[f"{m}_ob"]) + ow @ bvfull
        pkA[:, A_XT8:A_XT8 + 6 * N] = xt8.astype(np.float64)
        pkB[:, B_XI8:B_XI8 + 8 * N] = xi8.astype(np.float64)
        pkD[:, D_RWH_I:D_RWH_I + 1024] = _pm(ip_w[sl].T * s8)
        pkD[:, D_RWH_T:D_RWH_T + 768] = _pm(tp_w[sl].T * s8)

        im = {"pkA": _q8(pkA), "pkB": _q8(pkB), "pkD": _q8(pkD),
              "pkC": _host_cd(pkC, cd),
              "colf": np.ascontiguousarray(colf, dtype=np.float32)}
        # exact fp8 bytes for the activations (avoid double-quantization)
        im["pkA"][:, A_XT8:A_XT8 + 6 * N] = xt8
        im["pkB"][:, B_XI8:B_XI8 + 8 * N] = xi8
        in_maps.append(im)

    res = run_bass_kernel_spmd(nc, in_maps, list(range(NCORES)))
    LAST_RESULT["res"] = res

    outs = []
    for mi, m in enumerate(("ia", "ta")):
        acc = np.zeros((DM, N), dtype=np.float64)
        for h in range(NCORES):
            acc += res.results[h][f"po_{m}"].astype(np.float64)
        acc /= PO_SCALE
        for h in range(NCORES):
            acc[h * DH:(h + 1) * DH] += \
                res.results[h]["xr"][:, mi, :].astype(np.float64)
        acc += ob_eff[m][:, None]
        outs.append(np.ascontiguousarray(acc.T[None]).astype(np.float32))
    return outs[0], outs[1]


# revision 12
# speedup vs baseline: 1.0289x; 1.0289x over previous
"""Trainium2 Bass kernel for nn_CrossModalFusionModel (sparse sliding-window
cross-attention, 2 modules: image<-text and text<-image).

Sharding: head-parallel tensor parallelism over 8 NeuronCores. Core h owns
attention head h (dh=128) of BOTH modules: it computes its head's Q/K/V with
host-folded projection weights (input-proj and attention-proj chains collapse
into one matmul), runs banded attention for that head, and emits a full-D
o-projection partial plus its D-slice of the residual projection. The host
sums the 8 partials (the unshard step). No collectives.

Cost-model-driven structure (v2 cost model):
- DMA issue is expensive (625ns HWDGE + 650ns SEQ hold per DMA, all DMAs
  serialize on one HWDGE device and one DMA_ENGINES device): inputs are
  packed into 4 dram tensors loaded by ~6 slice-DMAs on the SP queue, small
  consts ride the Pool SWDGE queue.
- po partials are emitted in fp8e4 (x16 scale): halves output bytes; the
  8-way host sum tolerates the quantization.
- Residual is a single hi-fp8 pass (x16-scaled weights); the lo-term
  corrections are dropped (error budget covers it).
- V is projected directly into [key, dh] layout with narrow DoubleRow
  matmuls per key-tile (no PE transposes); the v-bias moves to the host
  output bias since softmax weights sum to 1 (zero-pad slots included).
- ssum/AV accumulation opens PSUM with disjoint start=True groups
  (c0 covers q[0:256), c3 covers q[256:512)): no opener matmuls.
- oproj matmul pairs write f16 PSUM banks [128, 2N]; single wide copies
  (balanced across Act/DVE/Pool) cast to fp8 stages that stream out from
  mid-kernel instead of piling into a tail.
- PE warmup matmuls seeded from an on-chip memset burn the cost model's
  pe-ramp window while the first DMAs land.
"""

import math

import numpy as np
import ml_dtypes

N = 512          # tokens / patches
DM = 1024        # d_model
DH = 128         # head dim
NT = N // 128    # 4 tiles
C_IMG = 1024
C_TXT = 768
WINDOW = 64
NCORES = 8

# per key-tile c, the consecutive query tiles it serves
CQTS = [[0, 1], [0, 1, 2], [1, 2, 3], [2, 3]]
GW = [len(q) * 128 for q in CQTS]            # group widths
GBASE = np.cumsum([0] + GW).tolist()
NG = GBASE[-1]                               # 1280

COMPUTE_DTYPE = "f16"
WARMUP_MM = 7
FP8_SCALE = 16.0    # folded into wq/wk/wv/rw (unfolded via Exp scale / host)
PO_SCALE = 16.0     # extra scale on fp8 po partials (folded into wo)

_prog_cache = {}
LAST_RESULT = {}

# cpack layout (CD [128, CP_LEN]); col-major consts then partition-0 rows
CP_MASK3 = 0                       # [128, 384] band masks [d+1 | d0 | d-1]
CP_COLC = 384                      # kbc_ia | kbc_ta | ones_col
CP_MASKP = 387                     # row [1, 512] at partition 0: npad counts
CP_ONESR = CP_MASKP + N            # row [1, 128] of ones
CP_LEN = CP_ONESR + DH
# tC layout (CD [128, TC_LEN]): cpack | wo_ia | wo_ta
TC_WO = {"ia": CP_LEN, "ta": CP_LEN + DM}
TC_LEN = CP_LEN + 2 * DM
# pkA (fp8 [128, A_LEN]): wq_ta | xt8 | wk_ia | wv_ia
A_WQ_TA, A_XT8, A_WK_IA, A_WV_IA = 0, 768, 3840, 4608
A_LEN = 5376
A_SPLIT = 1792            # first DMA: wq_ta + xt8 chunks 0-1
# pkB (fp8 [128, B_LEN]): wq_ia | xi8 | wk_ta | wv_ta
B_WQ_IA, B_XI8, B_WK_TA, B_WV_TA = 0, 1024, 5120, 6144
B_LEN = 7168
B_SPLIT = 3072            # first DMA: wq_ia + xi8 chunks 0-3
# pkD (fp8 [128, D_LEN]): rwh_i | rwh_t | rwl_i | rwl_t
D_RWH_I, D_RWH_T, D_RWL_I, D_RWL_T = 0, 1024, 1792, 2816
D_LEN = 3584
# pkE (fp8 [128, E_LEN]): xl8_i | xl8_t (lo-fp8 residual corrections)
E_XL_I, E_XL_T = 0, 4096
E_LEN = 7168
# colf (f32 [128, 8]): bq_ia bk_ia bq_ta bk_ta brx brt - -


def _np_cd(cd):
    return {"f16": np.float16, "bf16": ml_dtypes.bfloat16}[cd]


def _host_cd(x, cd):
    return np.ascontiguousarray(np.ascontiguousarray(x).astype(_np_cd(cd)))


def _q8(x):
    return np.ascontiguousarray(
        np.asarray(x, dtype=np.float32).astype(ml_dtypes.float8_e4m3))


def _pm(w):
    """[C, X] -> partition-major [128, (C//128)*X]."""
    C, X = w.shape
    return (np.ascontiguousarray(w).reshape(C // 128, 128, X)
            .transpose(1, 0, 2).reshape(128, (C // 128) * X))


def _build_program(cd):
    import concourse.bass as bass
    import concourse.tile as tile
    from concourse import bacc, mybir

    f32 = mybir.dt.float32
    CD = {"f16": mybir.dt.float16, "bf16": mybir.dt.bfloat16}[cd]
    CD8 = mybir.dt.float8e4
    Exp = mybir.ActivationFunctionType.Exp
    Copy = mybir.ActivationFunctionType.Copy
    Ident = mybir.ActivationFunctionType.Identity
    DR = mybir.MatmulPerfMode.DoubleRow

    nc = bacc.Bacc("TRN2", target_bir_lowering=False, debug=False,
                   num_devices=NCORES)

    d_pkA = nc.dram_tensor("pkA", [128, A_LEN], CD8, kind="ExternalInput")
    d_pkB = nc.dram_tensor("pkB", [128, B_LEN], CD8, kind="ExternalInput")
    d_pkC = nc.dram_tensor("pkC", [128, TC_LEN], CD, kind="ExternalInput")
    d_pkD = nc.dram_tensor("pkD", [128, D_LEN], CD8, kind="ExternalInput")
    d_pkE = nc.dram_tensor("pkE", [128, E_LEN], CD8, kind="ExternalInput")
    d_colf = nc.dram_tensor("colf", [128, 8], f32, kind="ExternalInput")

    d_po = {m: nc.dram_tensor(f"po_{m}", [DM, N], CD8, kind="ExternalOutput")
            for m in ("ia", "ta")}
    d_xr = nc.dram_tensor("xr", [128, 2, N], CD, kind="ExternalOutput")

    DESCALE = 1.0 / (FP8_SCALE * FP8_SCALE)

    with tile.TileContext(nc) as tc:
        with tc.tile_pool(name="consts", bufs=1) as consts, \
             tc.tile_pool(name="work", bufs=2) as work, \
             tc.tile_pool(name="ps_big", bufs=4, space="PSUM") as ps_big, \
             tc.tile_pool(name="ps_po", bufs=2, space="PSUM") as ps_po:

            # ---- PE warmup seeded from an on-chip memset (no DMA dep).
            # The dummy Exp forces the ACT table load (1283ns) into the
            # idle startup window.
            seed = consts.tile([128, N], CD, tag="seed")
            nc.gpsimd.memset(seed[:], 0.0)
            dummy = consts.tile([1, N], CD, tag="dummy")
            nc.scalar.activation(dummy[:], seed[0:1, :], Exp)
            warm_ps = ps_big.tile([128, N], f32, tag="big", name="warm_ps")
            for _ in range(WARMUP_MM):
                nc.tensor.matmul(warm_ps[:], seed[0:1, 0:128], dummy[:],
                                 start=True, stop=True)

            # ---- DMAs: packed inputs, slice loads in dependency order ----
            tA = consts.tile([128, A_LEN], CD8, tag="tA", name="tA")
            tB = consts.tile([128, B_LEN], CD8, tag="tB", name="tB")
            tC = consts.tile([128, TC_LEN], CD, tag="tC", name="tC")
            tD = consts.tile([128, D_LEN], CD8, tag="tD", name="tD")
            tE = consts.tile([128, E_LEN], CD8, tag="tE", name="tE")
            colf = consts.tile([128, 8], f32, tag="colf")

            nc.sync.dma_start(tA[:, 0:A_SPLIT], d_pkA[:, 0:A_SPLIT])
            nc.sync.dma_start(tA[:, A_SPLIT:], d_pkA[:, A_SPLIT:])
            nc.sync.dma_start(tC[:, 0:CP_LEN], d_pkC[:, 0:CP_LEN])
            nc.sync.dma_start(tB[:, 0:B_SPLIT], d_pkB[:, 0:B_SPLIT])
            nc.sync.dma_start(tB[:, B_SPLIT:], d_pkB[:, B_SPLIT:])
            nc.sync.dma_start(tC[:, CP_LEN:], d_pkC[:, CP_LEN:])
            nc.sync.dma_start(tE[:], d_pkE[:])
            nc.gpsimd.dma_start(colf[:], d_colf[:])
            nc.gpsimd.dma_start(tD[:], d_pkD[:])

            def c3(t, off, nct):
                return t[:, off:off + nct * 128].rearrange(
                    "p (c x) -> p c x", x=128)

            def x3(t, off, nct):
                return t[:, off:off + nct * N].rearrange(
                    "p (c x) -> p c x", x=N)

            wq = {"ia": c3(tB, B_WQ_IA, 8), "ta": c3(tA, A_WQ_TA, 6)}
            wk = {"ia": c3(tA, A_WK_IA, 6), "ta": c3(tB, B_WK_TA, 8)}
            wv = {"ia": c3(tA, A_WV_IA, 6), "ta": c3(tB, B_WV_TA, 8)}
            xt8 = x3(tA, A_XT8, 6)
            xi8 = x3(tB, B_XI8, 8)
            rwh = {"i": c3(tD, D_RWH_I, 8), "t": c3(tD, D_RWH_T, 6)}
            rwl = {"i": c3(tD, D_RWL_I, 8), "t": c3(tD, D_RWL_T, 6)}
            xl8 = {"i": x3(tE, E_XL_I, 8), "t": x3(tE, E_XL_T, 6)}
            wo = {m: tC[:, TC_WO[m]:TC_WO[m] + DM] for m in ("ia", "ta")}
            mask3 = tC[:, CP_MASK3:CP_MASK3 + 384]
            kbc = {"ia": tC[:, CP_COLC:CP_COLC + 1],
                   "ta": tC[:, CP_COLC + 1:CP_COLC + 2]}
            ones_col = tC[:, CP_COLC + 2:CP_COLC + 3]
            maskP = tC[0:1, CP_MASKP:CP_MASKP + N]
            ones_row = tC[0:1, CP_ONESR:CP_ONESR + DH]
            bq = {"ia": colf[:, 0:1], "ta": colf[:, 2:3]}
            bk = {"ia": colf[:, 1:2], "ta": colf[:, 3:4]}
            brx = colf[:, 4:5]
            brt = colf[:, 5:6]

            st = {"ia": {}, "ta": {}}

            def projT(m, which, w3, xx, nct, tag, evac):
                """[128, N] = (w^T x)^T via fp8 DR chunks; bias in evac.
                evac: 'a' = Act Ident+bias, 'v' = DVE tensor_scalar_add."""
                ps = ps_big.tile([128, N], f32, tag="big")
                for c in range(nct // 2):
                    nc.tensor.matmul(ps[:], w3[:, 2 * c:2 * c + 2, :],
                                     xx[:, 2 * c:2 * c + 2, :],
                                     perf_mode=DR, start=(c == 0),
                                     stop=(c == nct // 2 - 1))
                out = work.tile([128, N], CD, tag=tag, bufs=1, name=tag)
                col = bq[m] if which == "q" else bk[m]
                if evac == "a":
                    nc.scalar.activation(out[:], ps[:], Ident, bias=col)
                else:
                    nc.vector.tensor_scalar_add(out[:], ps[:], col[:])
                st[m][which] = out
                return out

            def vproj(m, xx, nct):
                """vN [key%128, c*DH+dh] direct: narrow DR matmuls per
                key-tile into one f16 PSUM tile; no bias (host-folded)."""
                vps = ps_big.tile([128, NT * DH], f32, tag="big", name="vps")
                for c in range(NT):
                    for j in range(nct // 2):
                        nc.tensor.matmul(
                            vps[:, c * DH:(c + 1) * DH],
                            xx[:, 2 * j:2 * j + 2, c * 128:(c + 1) * 128],
                            wv[m][:, 2 * j:2 * j + 2, :],
                            perf_mode=DR, start=(j == 0),
                            stop=(j == nct // 2 - 1))
                vN = work.tile([128, NT * DH], CD, tag=f"vN_{m}", bufs=1,
                               name=f"vN_{m}")
                nc.vector.tensor_copy(vN[:], vps[:])
                st[m]["vN"] = vN

            def attn_scores(m):
                """Banded scores by key-tile group -> exp -> mask."""
                qT, kT = st[m]["q"], st[m]["k"]
                eTm = work.tile([128, NG], CD, tag=f"eTm_{m}", bufs=1,
                                name=f"eTm_{m}")
                eT = work.tile([128, NG], CD, tag=f"eT_{m}", bufs=1,
                               name=f"eT_{m}")
                for c in range(NT):
                    qts = CQTS[c]
                    gps = ps_big.tile([128, GW[c]], f32, tag="big", name="gps")
                    nc.tensor.matmul(
                        gps[:], kT[:, c * 128:(c + 1) * 128],
                        qT[:, qts[0] * 128:(qts[-1] + 1) * 128],
                        start=True, stop=True)
                    sl = slice(GBASE[c], GBASE[c + 1])
                    nc.scalar.activation(eT[:, sl], gps[:], Exp,
                                         scale=DESCALE)
                    moff = (1 - (c - qts[0])) * 128
                    nc.vector.tensor_mul(eTm[:, sl], eT[:, sl],
                                         mask3[:, moff:moff + GW[c]])
                sp = ps_big.tile([128, N], f32, tag="big", name="sp")[0:1, :]
                nc.tensor.matmul(sp[:], kbc[m], qT[:], start=True, stop=True)
                eP = work.tile([1, N], CD, tag=f"eP_{m}", bufs=1,
                               name=f"eP_{m}")
                nc.scalar.activation(eP[:], sp[:], Exp, scale=DESCALE)
                ePm = work.tile([1, N], CD, tag=f"ePm_{m}", bufs=1,
                                name=f"ePm_{m}")
                nc.vector.tensor_mul(ePm[:], eP[:], maskP)
                st[m].update(eTm=eTm, ePm=ePm)

            # disjoint start groups: c0 covers q[0:256), c3 covers [256:512)
            GORDER = [0, 3, 1, 2]

            def attn_ssum(m):
                eTm, ePm = st[m]["eTm"], st[m]["ePm"]
                ssum = ps_big.tile([128, N], f32, tag="big", name="ssum")[0:1, :]
                for i, c in enumerate(GORDER):
                    qts = CQTS[c]
                    nc.tensor.matmul(
                        ssum[:, qts[0] * 128:(qts[-1] + 1) * 128],
                        ones_col, eTm[:, GBASE[c]:GBASE[c + 1]],
                        start=(i < 2), stop=False, skip_group_check=True)
                nc.tensor.matmul(ssum[:], ones_col[0:1, :], ePm[:],
                                 start=False, stop=True, skip_group_check=True)
                rinv = work.tile([1, N], CD, tag=f"rinv_{m}", bufs=1,
                                 name=f"rinv_{m}")
                with nc.allow_low_precision(
                        reason="1/denom feeds a 16-bit matmul; denom O(10-100)"):
                    nc.vector.reciprocal(rinv[:], ssum[:])
                st[m]["rinv"] = rinv

            def attn_rbc(m):
                rps = ps_big.tile([128, N], f32, tag="big", name="rps")
                nc.tensor.matmul(rps[:], ones_row, st[m]["rinv"][:],
                                 start=True, stop=True)
                rbc = work.tile([128, N], CD, tag=f"rbc_{m}", bufs=1,
                                name=f"rbc_{m}")
                nc.gpsimd.tensor_copy(rbc[:], rps[:])
                st[m]["rbc"] = rbc

            def attn_av(m):
                """oT [128, N] = V^T E^T; pad slots contribute zero (v-bias
                host-folded), so no pad matmul."""
                eTm, vN = st[m]["eTm"], st[m]["vN"]
                oT = ps_big.tile([128, N], f32, tag="big", name="oT")
                for i, c in enumerate(GORDER):
                    qts = CQTS[c]
                    nc.tensor.matmul(
                        oT[:, qts[0] * 128:(qts[-1] + 1) * 128],
                        vN[:, c * DH:(c + 1) * DH],
                        eTm[:, GBASE[c]:GBASE[c + 1]],
                        start=(i < 2), stop=(i == NT - 1),
                        skip_group_check=True)
                onorm = work.tile([128, N], CD, tag=f"onorm_{m}", bufs=1,
                                  name=f"onorm_{m}")
                nc.vector.tensor_mul(onorm[:], oT[:], st[m]["rbc"][:])
                st[m]["onorm"] = onorm

            def oproj_pair(m, p, eng):
                """chunks 2p,2p+1 -> one f16 PSUM bank -> one wide copy into
                the fp8 stage tile. eng in {'v','a','p'}."""
                onorm = st[m]["onorm"]
                pps = ps_po.tile([128, 2 * N], f32, tag="po", name="pps")
                for k in range(2):
                    dt_i = 2 * p + k
                    nc.tensor.matmul(pps[:, k * N:(k + 1) * N],
                                     wo[m][:, dt_i * 128:(dt_i + 1) * 128],
                                     onorm[:], start=True, stop=True)
                half = p // 2
                stage = st[m]["stage"][half]
                dst = stage[:, (p % 2) * 2:(p % 2) * 2 + 2, :].rearrange(
                    "p a b -> p (a b)")
                if eng == "v":
                    nc.vector.tensor_copy(dst, pps[:])
                elif eng == "a":
                    nc.scalar.activation(dst, pps[:], Copy)
                else:
                    nc.gpsimd.tensor_copy(dst, pps[:])

            def po_flush(m, half):
                stage = st[m]["stage"][half]
                nc.sync.dma_start(
                    d_po[m].ap().rearrange("(c p) n -> p c n", p=128)
                    [:, half * 4:half * 4 + 4, :], stage[:])

            for m in ("ia", "ta"):
                st[m]["stage"] = [
                    work.tile([128, 4, N], CD8, tag=f"stg_{m}{h}", bufs=1,
                              name=f"stg_{m}{h}") for h in range(2)]
            xr_sb = work.tile([128, 2, N], CD, tag="xr_sb", bufs=1,
                              name="xr_sb")

            def resid_mm(which, ps, r0, r1):
                """resid*16 = xh@wh + xl@wh + xh@wl, fp8 DR passes; flat
                pass index r in [0, 3*nct//2)."""
                nct = 8 if which == "i" else 6
                xx = xi8 if which == "i" else xt8
                passes = [(rwh[which], xx), (rwh[which], xl8[which]),
                          (rwl[which], xx)]
                nh = nct // 2
                for r in range(r0, r1):
                    w3, x3_ = passes[r // nh]
                    c = r % nh
                    nc.tensor.matmul(ps[:], w3[:, 2 * c:2 * c + 2, :],
                                     x3_[:, 2 * c:2 * c + 2, :], perf_mode=DR,
                                     start=(r == 0), stop=(r == 3 * nh - 1),
                                     skip_group_check=True)

            # ---- schedule (program order == per-engine issue order) ----
            # ia chain runs start-to-finish as early as possible; ta chain
            # and residual passes fill PE gaps; outputs stream in readiness
            # order on the SP queue.
            projT("ta", "q", wq["ta"], xt8, 6, "qta", "v")
            projT("ia", "k", wk["ia"], xt8, 6, "kia", "a")
            vproj("ia", xt8, 6)
            rx2 = ps_po.tile([128, 2 * N], f32, tag="po", name="rx2")
            resid_mm("t", rx2[:, 0:N], 0, 3)        # t hi pass (PE gap)
            projT("ia", "q", wq["ia"], xi8, 8, "qia", "v")
            attn_scores("ia")
            projT("ta", "k", wk["ta"], xi8, 8, "kta", "a")
            vproj("ta", xi8, 8)
            resid_mm("i", rx2[:, N:], 0, 4)         # i hi pass (PE gap)
            attn_ssum("ia")
            attn_rbc("ia")
            attn_scores("ta")
            attn_av("ia")
            resid_mm("t", rx2[:, 0:N], 3, 9)        # lo passes (need tE)
            resid_mm("i", rx2[:, N:], 4, 12)
            nc.scalar.activation(xr_sb[:, 1, :], rx2[:, 0:N], Ident,
                                 bias=brt, scale=1.0 / FP8_SCALE)
            nc.scalar.activation(xr_sb[:, 0, :], rx2[:, N:], Ident,
                                 bias=brx, scale=1.0 / FP8_SCALE)
            nc.sync.dma_start(d_xr[:], xr_sb[:])
            attn_ssum("ta")
            attn_rbc("ta")
            oproj_pair("ia", 0, "a")
            oproj_pair("ia", 1, "v")
            attn_av("ta")
            oproj_pair("ia", 2, "p")
            oproj_pair("ia", 3, "a")
            po_flush("ia", 0)
            oproj_pair("ta", 0, "v")
            oproj_pair("ta", 1, "p")
            po_flush("ia", 1)
            oproj_pair("ta", 2, "a")
            po_flush("ta", 0)
            oproj_pair("ta", 3, "v")
            po_flush("ta", 1)

    nc.compile()
    return nc


def _band_masks():
    """[128, 3*128] = [d+1 | d0 | d-1]; pattern for key tile c vs query
    tile qt is delta = c - qt, stored so a key-group's consecutive query
    tiles read one contiguous slice."""
    jj = np.arange(128)[:, None]
    ii = np.arange(128)[None, :]
    out = np.zeros((128, 3 * 128), dtype=np.float64)
    for i, d in enumerate((1, 0, -1)):
        delta = 128 * d + jj - ii
        out[:, i * 128:(i + 1) * 128] = ((delta >= -WINDOW // 2)
                                         & (delta <= WINDOW // 2 + 1))
    return out


def _npad():
    i = np.arange(N)
    lo = np.maximum(0, i - WINDOW // 2)
    hi = np.minimum(N - 1, i + WINDOW // 2 + 1)
    length = hi - lo + 1
    return np.maximum(0, WINDOW - length).astype(np.float64)


def kernel(**inputs):
    from concourse.bass_utils import run_bass_kernel_spmd

    cd = COMPUTE_DTYPE
    if cd not in _prog_cache:
        _prog_cache[cd] = _build_program(cd)
    nc = _prog_cache[cd]

    f8 = lambda x: np.asarray(x, dtype=np.float64)
    images = f8(inputs["images"])[0]        # [N, 1024]
    caps = f8(inputs["capitions"])[0]       # [N, 768]
    ip_w, ip_b = f8(inputs["ip_w"]), f8(inputs["ip_b"])
    tp_w, tp_b = f8(inputs["tp_w"]), f8(inputs["tp_b"])

    sc = 1.0 / math.sqrt(DH)
    s8 = FP8_SCALE
    band = _band_masks()
    xi_pm = _pm(images.T)                   # [128, 8*N]
    xt_pm = _pm(caps.T)                     # [128, 6*N]
    xi8, xt8 = _q8(xi_pm), _q8(xt_pm)
    pkE8 = np.zeros((128, E_LEN), dtype=ml_dtypes.float8_e4m3)
    pkE8[:, E_XL_I:E_XL_I + 8 * N] = _q8(xi_pm - xi8.astype(np.float64))
    pkE8[:, E_XL_T:E_XL_T + 6 * N] = _q8(xt_pm - xt8.astype(np.float64))
    pkE8 = np.ascontiguousarray(pkE8)

    in_maps = []
    ob_eff = {}
    for h in range(NCORES):
        sl = slice(h * DH, (h + 1) * DH)
        pkA = np.zeros((128, A_LEN), dtype=np.float64)
        pkB = np.zeros((128, B_LEN), dtype=np.float64)
        pkC = np.zeros((128, TC_LEN), dtype=np.float64)
        pkD = np.zeros((128, D_LEN), dtype=np.float64)
        colf = np.zeros((128, 8), dtype=np.float64)

        pkC[:, CP_MASK3:CP_MASK3 + 384] = band
        pkC[:, CP_COLC + 2] = 1.0
        pkC[0, CP_MASKP:CP_MASKP + N] = _npad()
        pkC[0, CP_ONESR:CP_ONESR + DH] = 1.0
        colf[:, 4] = ip_b[sl]
        colf[:, 5] = tp_b[sl]

        for mi, (m, pw, pb, cw, cb) in enumerate(
                (("ia", ip_w, ip_b, tp_w, tp_b),
                 ("ta", tp_w, tp_b, ip_w, ip_b))):
            qw, qb = f8(inputs[f"{m}_qw"]), f8(inputs[f"{m}_qb"])
            kw, kb = f8(inputs[f"{m}_kw"]), f8(inputs[f"{m}_kb"])
            vw, vb = f8(inputs[f"{m}_vw"]), f8(inputs[f"{m}_vb"])
            ow = f8(inputs[f"{m}_ow"])
            wq_p = _pm(((qw[sl] @ pw) * sc * s8).T)
            wk_p = _pm(((kw[sl] @ cw) * s8).T)
            wv_p = _pm(((vw[sl] @ cw) * s8).T)
            if m == "ia":
                pkA[:, A_WK_IA:A_WK_IA + 768] = wk_p
                pkA[:, A_WV_IA:A_WV_IA + 768] = wv_p
                pkB[:, B_WQ_IA:B_WQ_IA + 1024] = wq_p
            else:
                pkA[:, A_WQ_TA:A_WQ_TA + 768] = wq_p
                pkB[:, B_WK_TA:B_WK_TA + 1024] = wk_p
                pkB[:, B_WV_TA:B_WV_TA + 1024] = wv_p
            pkC[:, TC_WO[m]:TC_WO[m] + DM] = ow[:, sl].T * (PO_SCALE / s8)
            colf[:, 2 * mi] = (qw[sl] @ pb + qb[sl]) * sc * s8
            colf[:, 2 * mi + 1] = (kw[sl] @ cb + kb[sl]) * s8
            pkC[:, CP_COLC + mi] = kb[sl] * s8
            if h == 0:
                # v-bias folds into the output bias (softmax weights sum
                # to 1, zero-pad slots included)
                bvfull = vw @ cb + vb
                ob_eff[m] = f8(inputs[f"{m}_ob"]) + ow @ bvfull
        rwi = _pm(ip_w[sl].T * s8)
        rwt = _pm(tp_w[sl].T * s8)
        rwi_h, rwt_h = _q8(rwi), _q8(rwt)
        pkD8 = np.zeros((128, D_LEN), dtype=ml_dtypes.float8_e4m3)
        pkD8[:, D_RWH_I:D_RWH_I + 1024] = rwi_h
        pkD8[:, D_RWH_T:D_RWH_T + 768] = rwt_h
        pkD8[:, D_RWL_I:D_RWL_I + 1024] = _q8(rwi - rwi_h.astype(np.float64))
        pkD8[:, D_RWL_T:D_RWL_T + 768] = _q8(rwt - rwt_h.astype(np.float64))

        im = {"pkA": _q8(pkA), "pkB": _q8(pkB),
              "pkD": np.ascontiguousarray(pkD8),
              "pkC": _host_cd(pkC, cd),
              "colf": np.ascontiguousarray(colf, dtype=np.float32)}
        # exact fp8 bytes for the activations (avoid double-quantization)
        im["pkA"][:, A_XT8:A_XT8 + 6 * N] = xt8
        im["pkB"][:, B_XI8:B_XI8 + 8 * N] = xi8
        im["pkE"] = pkE8
        in_maps.append(im)

    res = run_bass_kernel_spmd(nc, in_maps, list(range(NCORES)))
    LAST_RESULT["res"] = res

    outs = []
    for mi, m in enumerate(("ia", "ta")):
        acc = np.zeros((DM, N), dtype=np.float64)
        for h in range(NCORES):
            acc += res.results[h][f"po_{m}"].astype(np.float64)
        acc /= PO_SCALE
        for h in range(NCORES):
            acc[h * DH:(h + 1) * DH] += \
                res.results[h]["xr"][:, mi, :].astype(np.float64)
        acc += ob_eff[m][:, None]
        outs.append(np.ascontiguousarray(acc.T[None]).astype(np.float32))
    return outs[0], outs[1]
